# revision 37
# baseline (speedup 1.0000x reference)
"""Trainium2 Bass kernel for sparse attention with relation bias.

Computes, for inputs (B=4, N=512, C=128, H=8, HS=16):
  qkv = joint @ W_qkv^T -> q,k,v
  attn = softmax((q k^T + rel @ W_r^T) * conn * HS^-0.5)
  out  = (attn @ v) @ W_proj^T + b_proj

Sharding: 8 cores, core i handles batch b=i//2 and n-row half i%2 (256 rows).
No collectives — each core computes its own output rows; host gathers.

Layout notes:
- All inputs are host-cast to bf16. rel is loaded with PAIRED m-rows: each
  512B DMA line carries the c-vectors of rows (2q, 2q+1), so bf16 keeps
  full line rate with half the bytes/descriptors of f32. The resulting
  m-permutation (evens then odds per 256-row half) is compensated on the
  host by permuting the m-axis of jointT (k/v) and conn identically —
  softmax over m is permutation invariant.
- Heads (HS=16) are padded to 32 partitions and split into two half-tensors
  (heads 0-3 "A", heads 4-7 "B") for q/k/v/x. Weights are host-padded.
- Attention PSUM layout per subgroup of 4 n-rows: partition = jj*32 + h.
- The 4 relation-bias matmuls per subgroup are issued back-to-back into
  distinct 32-wide column groups (tile_position) so they run concurrently.
- Queues: rel loads + Qexp strips on gpsimd(SWDGE), conn broadcasts on
  sync(HWDGE), output stores on scalar(HWDGE) — keeps consumer-gated DMAs
  from head-of-line-blocking the rel stream.
"""

import sys

sys.path.insert(0, "/opt/trn_rl_repo")

import numpy as np
import ml_dtypes

import concourse.bass as bass
import concourse.tile as tile
from concourse import bacc, mybir
from concourse.masks import make_identity
from contextlib import ExitStack

F32 = mybir.dt.float32
BF16 = mybir.dt.bfloat16
F8 = mybir.dt.float8e4
BF16NP = ml_dtypes.bfloat16
F8NP = ml_dtypes.float8_e4m3fn

# Problem constants (hardcoded per spec)
B, N, C, H = 4, 512, 128, 8
HS = C // H  # 16
SCALE = float(HS) ** -0.5
NCORES = 8
P = 128  # partitions
MC = N // P  # m-chunks per row = 4
HH = H // 2  # heads per half = 4

# wqkvT_pad column sections: qA qB kA kB vA vB, each 128 wide
QA, QB, KA, KB, VA, VB = (i * P for i in range(6))


def build_graph(NH, G=16):
    """Build the SPMD single-core graph. NH = n-rows per core."""
    NG = NH // G  # groups of 16 n-rows

    nc = bacc.Bacc("TRN2", target_bir_lowering=False, debug=False)
    rel_d = nc.declare_dram_parameter("rel", [NH * N, C], F8, isOutput=False)
    conn_d = nc.declare_dram_parameter("conn", [NH, N], BF16, isOutput=False)
    jT_d = nc.declare_dram_parameter("jointT", [C, N], BF16, isOutput=False)
    jTq_d = nc.declare_dram_parameter("jointTq", [C, NH], BF16, isOutput=False)
    wqkvT_d = nc.declare_dram_parameter("wqkvT", [C, 6 * P], BF16, isOutput=False)
    wrS_d = nc.declare_dram_parameter("wrS", [C, 8 * P], F8, isOutput=False)
    wpT_d = nc.declare_dram_parameter("wprojT", [C, 2 * C], BF16, isOutput=False)
    bp_d = nc.declare_dram_parameter("bproj", [1, C], BF16, isOutput=False)
    out_d = nc.declare_dram_parameter("out", [NH, C], F32, isOutput=True)

    with tile.TileContext(nc) as tc, ExitStack() as ctx:
        singles = ctx.enter_context(tc.tile_pool(name="singles", bufs=1))
        relqp = ctx.enter_context(tc.tile_pool(name="relqp", bufs=8))
        relTsbp = ctx.enter_context(tc.tile_pool(name="relTsbp", bufs=10))
        connp = ctx.enter_context(tc.tile_pool(name="connp", bufs=8))
        logitp = ctx.enter_context(tc.tile_pool(name="logitp", bufs=4))
        attnwp = ctx.enter_context(tc.tile_pool(name="attnwp", bufs=8))
        aTp = ctx.enter_context(tc.tile_pool(name="aTp", bufs=3))
        smallp = ctx.enter_context(tc.tile_pool(name="smallp", bufs=10))
        outp = ctx.enter_context(tc.tile_pool(name="outp", bufs=2))

        ps_attn = ctx.enter_context(tc.tile_pool(name="ps_attn", bufs=2, space="PSUM"))
        # relT and aT transposes share one 2-slot PSUM pool
        ps_relT = ctx.enter_context(tc.tile_pool(name="ps_relT", bufs=3, space="PSUM"))
        ps_aT = ps_relT
        ps_x = ctx.enter_context(tc.tile_pool(name="ps_x", bufs=2, space="PSUM"))
        ps_o = ctx.enter_context(tc.tile_pool(name="ps_o", bufs=1, space="PSUM"))

        # ---- constants / weights (all bf16 in DRAM; HWDGE loads) ----
        ident = singles.tile([P, P], BF16)
        make_identity(nc, ident)
        ident8 = singles.tile([P, P], F8)
        make_identity(nc, ident8)
        wqkvT = singles.tile([P, 6 * P], BF16)
        nc.sync.dma_start(out=wqkvT, in_=wqkvT_d[:, :])
        wrS = singles.tile([P, 8 * P], F8)
        nc.sync.dma_start(out=wrS, in_=wrS_d[:, :])
        # view: [c][ko=row-in-pair][pair][col]
        wrS2 = wrS.rearrange("p (r kp q) -> p r kp q", r=2, kp=4)
        wpT = singles.tile([P, 2 * C], BF16)
        nc.sync.dma_start(out=wpT, in_=wpT_d[:, :])
        bp = singles.tile([1, C], BF16)
        nc.sync.dma_start(out=bp, in_=bp_d[:, :])
        ones = singles.tile([1, G], BF16)
        nc.vector.memset(ones, 1.0)
        jT = singles.tile([P, N], BF16)
        nc.sync.dma_start(out=jT, in_=jT_d[:, :])
        jTq = singles.tile([P, NH], BF16)
        nc.sync.dma_start(out=jTq, in_=jTq_d[:, :])
        # double-buffered by group parity: group g+1's strips can be
        # written while group g's QK still reads the other buffer.
        QexpA0 = singles.tile([P, 2 * P], BF16)
        nc.vector.memset(QexpA0, 0.0)
        QexpA1 = singles.tile([P, 2 * P], BF16)
        nc.vector.memset(QexpA1, 0.0)
        QexpB0 = singles.tile([P, 2 * P], BF16)
        nc.vector.memset(QexpB0, 0.0)
        QexpB1 = singles.tile([P, 2 * P], BF16)
        nc.vector.memset(QexpB1, 0.0)
        xTA = singles.tile([P, G], BF16)
        nc.vector.memset(xTA, 0.0)
        xTB = singles.tile([P, G], BF16)
        nc.vector.memset(xTB, 0.0)

        kTA = singles.tile([P, N], BF16)
        kTB = singles.tile([P, N], BF16)
        vnatA = singles.tile([P, MC, P], BF16)
        vnatB = singles.tile([P, MC, P], BF16)
        qTA = singles.tile([P, NH], BF16)
        qTB = singles.tile([P, NH], BF16)

        # ---- prep: qkv projections (padded head layout, halves A/B) ----
        for dst, col in ((kTA, KA), (kTB, KB)):
            pk = ps_attn.tile([P, N], F32, tag="attn")
            nc.tensor.matmul(
                pk, lhsT=wqkvT[:, col : col + P], rhs=jT, start=True, stop=True
            )
            nc.vector.tensor_copy(dst, pk)
        for dst, col in ((vnatA, VA), (vnatB, VB)):
            for t in range(MC):
                pv = ps_attn.tile([P, N], F32, tag="attn")
                nc.tensor.matmul(
                    pv[:, :P],
                    lhsT=jT[:, t * P : (t + 1) * P],
                    rhs=wqkvT[:, col : col + P],
                    start=True,
                    stop=True,
                )
                nc.vector.tensor_copy(dst[:, t, :], pv[:, :P])
        for dst, col in ((qTA, QA), (qTB, QB)):
            pq = ps_attn.tile([P, N], F32, tag="attn")
            nc.tensor.matmul(
                pq[:, :NH], lhsT=wqkvT[:, col : col + P], rhs=jTq, start=True, stop=True
            )
            nc.vector.tensor_copy(dst, pq[:, :NH])

        # rotate big PSUM->SBUF copies across DVE/ACT to balance load.
        # On DVE, copy bf16 data as uint32 (half the element count — bf16 PSUM
        # sources get no perf-mode acceleration, u32 halves the stream).
        def copy_rot(i, out, in_):
            if i % 2 == 0:
                if (
                    out.dtype == in_.dtype
                    and out.dtype in (BF16, F8)
                    and in_.ap[-1][0] == 1
                    and out.ap[-1][0] == 1
                ):
                    nc.vector.tensor_copy(
                        out.bitcast(mybir.dt.uint32), in_.bitcast(mybir.dt.uint32)
                    )
                else:
                    nc.vector.tensor_copy(out, in_)
            else:
                nc.scalar.copy(out, in_)

        # rel viewed as quads of m-rows: DRAM row (t*128+p)*4+f, so
        # partition p of n-row t holds the c-vectors of m = 4p+f, f=0..3
        # (512B contiguous fp8 lines).
        rel_view = rel_d[:, :].rearrange("(t p four) c -> p t (four c)", p=P, four=4)

        for g in range(NG):
            relq_s = []
            for s in range(4):
                rq = relqp.tile([P, MC, 4 * C], F8, tag="relq")
                t0 = g * G + s * 4
                nc.gpsimd.dma_start(out=rq, in_=rel_view[:, t0 : t0 + MC, :])
                relq_s.append(rq)

            # Qexp strips: QexpX[hp*32+dh, s*128+jj*32+h] = qTX[hp*32+dh, n(s,jj)]
            QexpA = QexpA0 if g % 2 == 0 else QexpA1
            QexpB = QexpB0 if g % 2 == 0 else QexpB1
            qvA = QexpA.rearrange("p (b2 j r) -> p b2 j r", b2=2, r=16)
            qvB = QexpB.rearrange("p (b2 j r) -> p b2 j r", b2=2, r=16)
            for hp in range(HH):
                for qv, qTx, h in ((qvA, qTA, hp), (qvB, qTB, hp + HH)):
                    nc.gpsimd.tensor_copy(
                        out=qv[hp * 32 : hp * 32 + HS, :, :, h],
                        in_=qTx[
                            hp * 32 : hp * 32 + HS, g * G : (g + 1) * G
                        ].rearrange("p (b2 j) -> p b2 j", j=8),
                    )

            attn_ws = []
            for b2 in range(2):
                n0 = g * G + b2 * 8
                conn_e = connp.tile([P, N], BF16)
                conn_src = conn_d[n0 : n0 + 8, :]
                conn_bcast = bass.AP(
                    tensor=conn_src.tensor,
                    offset=conn_src.offset,
                    ap=[conn_src.ap[0], [0, 16], conn_src.ap[1]],
                )
                nc.sync.dma_start(out=conn_e[:, :], in_=conn_bcast)

                Pattn = ps_attn.tile([P, N], F32, tag="attn")
                nc.tensor.matmul(
                    Pattn,
                    lhsT=QexpA[:, b2 * P : (b2 + 1) * P],
                    rhs=kTA,
                    start=True,
                    stop=False,
                )
                nc.tensor.matmul(
                    Pattn,
                    lhsT=QexpB[:, b2 * P : (b2 + 1) * P],
                    rhs=kTB,
                    start=False,
                    stop=False,
                )

                # transpose the 8 n-rows' rel into [c, m] tiles (m
                # permuted), cast to fp8 pairs during the PSUM->SBUF copy
                relT_kp = []
                for j in range(8):
                    r16 = b2 * 8 + j
                    s4, jr = r16 // 4, r16 % 4
                    # fp8 transpose-mode writes need element step 2 (16-bit
                    # lanes); the pack to unit stride happens in the copy.
                    PT = ps_relT.tile([P, 2 * N], F8, tag="tp")
                    PTv = PT.rearrange("p (n two) -> p n two", two=2)
                    for blk in range(4):
                        nc.tensor.transpose(
                            PTv[:, blk * P : (blk + 1) * P, 0],
                            relq_s[s4][:, jr, blk * P : (blk + 1) * P],
                            ident8,
                        )
                    if j % 2 == 0:
                        relTpair = relTsbp.tile([P, 2, N], F8)
                        relT_kp.append(relTpair)
                    copy_rot(8 * b2 + j, relT_kp[j // 2][:, j % 2, :], PTv[:, :, 0])

                # relation bias: fp8 DoubleRow — one full-width matmul per
                # n-row PAIR (2 rhs values/cycle), with host-built scatter
                # stationaries landing W_r on partition (2kp+r)*16+h.
                for kp in range(4):
                    nc.tensor.matmul(
                        Pattn,
                        lhsT=wrS2[:, :, kp, :],
                        rhs=relT_kp[kp][:, :, :],
                        start=False,
                        stop=(kp == 3),
                        perf_mode=mybir.MatmulPerfMode.DoubleRow,
                    )

                logits = logitp.tile([P, N], BF16)
                nc.vector.tensor_mul(logits, Pattn, conn_e)
                attn_w = attnwp.tile([P, N], BF16)
                sums = smallp.tile([P, 1], F32)
                nc.scalar.activation(
                    out=attn_w,
                    in_=logits,
                    func=mybir.ActivationFunctionType.Exp,
                    scale=SCALE,
                    accum_out=sums,
                )
                recip = smallp.tile([P, 1], F32)
                nc.vector.reciprocal(recip, sums)
                nc.vector.tensor_scalar_mul(attn_w, attn_w, recip)
                attn_ws.append(attn_w)

            # attn @ v, accumulated over m-chunks; PXx free = b2*64 + j*8 + h
            PXA = ps_x.tile([P, P], F32, tag="px")
            PXB = ps_x.tile([P, P], F32, tag="px")
            for c in range(MC):
                aT = aTp.tile([P, 2 * P], BF16)
                PA = ps_aT.tile([P, N], BF16, tag="tp")
                for b2 in range(2):
                    nc.tensor.transpose(
                        PA[:, b2 * P : (b2 + 1) * P],
                        attn_ws[b2][:, c * P : (c + 1) * P],
                        ident,
                    )
                copy_rot(c, aT, PA[:, : 2 * P])
                # rhs skips the 8 pad columns per 16-block: free = 128
                aT_tight = aT.rearrange("p (b2 j r) -> p b2 j r", b2=2, r=16)[
                    :, :, :, 0:H
                ]
                nc.tensor.matmul(
                    PXA,
                    lhsT=vnatA[:, c, :],
                    rhs=aT_tight,
                    start=(c == 0),
                    stop=(c == MC - 1),
                )
                nc.tensor.matmul(
                    PXB,
                    lhsT=vnatB[:, c, :],
                    rhs=aT_tight,
                    start=(c == 0),
                    stop=(c == MC - 1),
                )

            # extract xTx[hp*32+dh, (s,jj)] = PXx[hp*32+dh, (4s+jj)*32 + h]
            pxvA = PXA.rearrange("p (q r) -> p q r", r=H)
            pxvB = PXB.rearrange("p (q r) -> p q r", r=H)
            for hp in range(HH):
                sl = slice(hp * 32, hp * 32 + HS)
                copy_rot(2 * hp, xTA[sl, :], pxvA[sl, :, hp])
                copy_rot(2 * hp + 1, xTB[sl, :], pxvB[sl, :, hp + HH])

            PO = ps_o.tile([G, C], F32)
            nc.tensor.matmul(PO, lhsT=xTA, rhs=wpT[:, :C], start=True, stop=False)
            nc.tensor.matmul(PO, lhsT=xTB, rhs=wpT[:, C:], start=False, stop=False)
            nc.tensor.matmul(PO, lhsT=ones, rhs=bp, start=False, stop=True)
            out_sb = outp.tile([G, C], F32)
            nc.vector.tensor_copy(out_sb, PO)
            nc.scalar.dma_start(out=out_d[g * G : (g + 1) * G, :], in_=out_sb)

    return nc


_GRAPH_CACHE = {}


def _get_graph(NH):
    if NH not in _GRAPH_CACHE:
        nc = build_graph(NH)
        nc.finalize()
        _GRAPH_CACHE[NH] = nc
    return _GRAPH_CACHE[NH]


def _pad_heads(W):
    """[H*HS, C] -> two padded [4*32, C] halves (heads 0-3, 4-7), zeros in pad rows."""
    Wr = W.reshape(H, HS, -1)
    out = []
    for half in range(2):
        pad = np.zeros((HH, 32, W.shape[-1]), dtype=W.dtype)
        pad[:, :HS] = Wr[half * HH : (half + 1) * HH]
        out.append(pad.reshape(HH * 32, -1))
    return out


def _bf(x):
    return np.ascontiguousarray(x.astype(BF16NP))


# m-permutation induced by the paired rel load: per 256-row half, evens
# then odds. k/v/conn m-axes are permuted to match on the host.
# column fi*128+p of a transposed rel tile holds m = 4p+fi
M_PERM = np.concatenate([np.arange(0, 512, 4) + fi for fi in range(4)])


def make_in_maps(joint_feature, relation_feature, conn_feature, W_qkv, W_r, W_proj, b_proj):
    """Shard full inputs into 8 per-core input maps (bf16 on host)."""
    NH = N // 2
    Wq, Wk, Wv = W_qkv[:C], W_qkv[C : 2 * C], W_qkv[2 * C :]
    qA, qB = _pad_heads(Wq)
    kA, kB = _pad_heads(Wk)
    vA, vB = _pad_heads(Wv)
    # wqkvT: [C, 6*128] — sections qA qB kA kB vA vB (transposed)
    wqkvT = _bf(np.concatenate([qA, qB, kA, kB, vA, vB], axis=0).T)
    # wprojT: [C(padded in-space per half), 2*C]
    WpT = W_proj.T  # [c_in, c_out]
    pA, pB = _pad_heads(WpT)  # pads c_in (= head space of x)
    wpT = _bf(np.concatenate([pA, pB], axis=1))
    wrS = np.zeros((C, 2, 4, P), dtype=np.float32)
    for kp in range(4):
        for r in range(2):
            q0 = (2 * kp + r) * 16
            wrS[:, r, kp, q0 : q0 + H] = W_r.T
    wrS = np.ascontiguousarray(wrS.reshape(C, 8 * P).astype(F8NP))
    bp = _bf(b_proj[None, :])
    rel_bf = relation_feature.astype(F8NP)
    conn_bf = conn_feature.astype(BF16NP)
    joint_bf = joint_feature.astype(BF16NP)
    in_maps = []
    for core in range(NCORES):
        b = core // 2
        half = core % 2
        n0 = half * NH
        jT = np.ascontiguousarray(joint_bf[b].T[:, M_PERM])
        jTq = np.ascontiguousarray(joint_bf[b, n0 : n0 + NH].T)
        rel = np.ascontiguousarray(rel_bf[b, n0 : n0 + NH].reshape(NH * N, C))
        conn = np.ascontiguousarray(conn_bf[b, n0 : n0 + NH][:, M_PERM])
        in_maps.append(
            {
                "rel": rel,
                "conn": conn,
                "jointT": jT,
                "jointTq": jTq,
                "wqkvT": wqkvT,
                "wrS": wrS,
                "wprojT": wpT,
                "bproj": bp,
            }
        )
    return in_maps


def kernel(joint_feature, relation_feature, conn_feature, W_qkv, W_r, W_proj, b_proj):
    from concourse.bass_utils import run_bass_kernel_spmd

    NH = N // 2
    nc = _get_graph(NH)
    in_maps = make_in_maps(
        joint_feature, relation_feature, conn_feature, W_qkv, W_r, W_proj, b_proj
    )
    res = run_bass_kernel_spmd(nc, in_maps, core_ids=list(range(NCORES)))
    out = np.zeros((B, N, C), dtype=np.float32)
    for core in range(NCORES):
        b = core // 2
        half = core % 2
        n0 = half * NH
        out[b, n0 : n0 + NH] = res.results[core]["out"]
    return out
